# revision 11
# baseline (speedup 1.0000x reference)
"""Multi-head attention forward on 8 Trainium2 NeuronCores.

Problem: nn_Attention_89060441850459
  inputs [8, 1024, 768] f32, w_qkv [768, 2304], w_proj [768, 768], b_proj [768]
  out = proj(softmax(q k^T / sqrt(64)) v) + b_proj,  H=12 heads, hd=64

Sharding: data parallel over batch — each of the 8 cores computes one batch
element end-to-end; weights replicated. No collectives.

Per-core dataflow (all matmuls in fp32r — fp32 operands truncated to fp22 in
the PE at 1 cycle/row for N>=256, fp32 PSUM accumulation):

  1. xT[d, n]   = PE-transpose of x[n, d]                       (d-major x)
  2. qkT[m, n]  = w_qkv[:, :1536].T @ xT      (q/k head-dim-major: [1536, 1024])
  3. v[n, c]    = x @ w_qkv[:, 1536:]          (s-major, heads padded with a
                  ones-column per head -> [1024, 12*65] so the PV matmul also
                  produces the softmax denominator for free)
  4. per head h, per key-chunk m (128 rows of kv):
       S^T_m [128k, 1024q] = kT_h[:, m].T-matmul  (lhsT=kT chunk, rhs=qT_h)
       E_m = exp(S^T_m / 8)                        (ACT, PSUM -> SBUF)
       O_aug[65, 1024q] += v_pad_m[:, h].T @ E_m   (accumulated over m in PSUM;
                                                    row 64 = sum_k E = Z)
     then O^T_h = O_aug[0:64] * broadcast(1/Z)     (DVE + DMA partition-bcast)
  5. y = O^T-stacked.T @ w_proj + b_proj           (lhsT = O^T d-major tiles)
"""

import sys

if "/opt/trn_rl_repo" not in sys.path:
    sys.path.insert(0, "/opt/trn_rl_repo")

from contextlib import ExitStack

import numpy as np

import concourse.bass as bass
import concourse.mybir as mybir
import concourse.tile as tile
from concourse import bacc
from concourse.masks import make_identity

B, N, D = 8, 1024, 768
H = 12
HD = D // H  # 64
NCORES = 8
P = 128
NT = N // P  # 8 seq chunks
DC = D // P  # 6 d chunks
F32 = mybir.dt.float32
F32R = mybir.dt.float32r
SCALE = HD**-0.5


def r(ap):
    """fp32 -> fp32r view for full-rate PE matmul."""
    return ap.bitcast(F32R)


def build_attention(ctx: ExitStack, tc: "tile.TileContext", x, w_qkv, w_proj, b_proj, y):
    nc = tc.nc
    exp = mybir.ActivationFunctionType.Exp

    perm = ctx.enter_context(tc.tile_pool(name="perm", bufs=1))
    psum = ctx.enter_context(tc.tile_pool(name="psum", bufs=2, space="PSUM"))

    identity = perm.tile([P, P], F32, tag="identity", name="identity")
    make_identity(nc, identity)

    # persistent SBUF arrays
    qkT = [perm.tile([P, N], F32R, tag=f"qkT{m}", name=f"qkT{m}") for m in range(12)]  # [0:6]=q, [6:12]=k
    vpad = [perm.tile([P, H * (HD + 1)], F32R, tag=f"vpad{i}", name=f"vpad{i}") for i in range(NT)]
    oT = [perm.tile([P, N], F32R, tag=f"oT{j}", name=f"oT{j}") for j in range(DC)]

    # ---------------- phase 1+2: xT, qkT, v ----------------
    with tc.tile_pool(name="tmp", bufs=1) as tmp, tc.tile_pool(name="xin", bufs=3) as xin:
        wq = [tmp.tile([P, 3 * D], F32R, tag=f"wq{k}", name=f"wq{k}") for k in range(DC)]
        for k in range(DC):
            nc.sync.dma_start(out=wq[k], in_=r(w_qkv[k * P : (k + 1) * P, :]))
        xT = [tmp.tile([P, N], F32R, tag=f"xT{j}", name=f"xT{j}") for j in range(DC)]

        for i in range(NT):
            xt = xin.tile([P, D], F32, tag="x", name="xt")
            nc.sync.dma_start(out=xt, in_=x[i * P : (i + 1) * P, :])
            for j in range(DC):
                pt = psum.tile([P, N], F32, tag="mm", name="mmps")
                nc.tensor.transpose(pt[:, 0:P], xt[:, j * P : (j + 1) * P], identity)
                nc.vector.tensor_copy(xT[j][:, i * P : (i + 1) * P], pt[:, 0:P])

        # qkT[m][dm, n] = sum_k w_qkv[k, m*128+dm] * xT[k, n]
        # (k outer so consecutive matmuls share the stationary operand)
        for m in range(12):
            ps = psum.tile([P, N], F32, tag="mm", name="mmps")
            for k in range(DC):
                for n2 in range(2):
                    nc.tensor.matmul(
                        ps[:, n2 * 512 : (n2 + 1) * 512],
                        lhsT=r(wq[k][:, m * P : (m + 1) * P]),
                        rhs=r(xT[k][:, n2 * 512 : (n2 + 1) * 512]),
                        start=(k == 0),
                        stop=(k == DC - 1),
                        skip_group_check=True,
                    )
            nc.vector.tensor_copy(qkT[m], ps)

        # v[i][n, c] = sum_k x[n, k] w_qkv[k, 1536+c], written head-padded
        for i in range(NT):
            ps = psum.tile([P, N], F32, tag="mm", name="mmps")
            for k in range(DC):
                for c0, cw in ((0, 512), (512, 256)):
                    nc.tensor.matmul(
                        ps[:, c0 : c0 + cw],
                        lhsT=r(xT[k][:, i * P : (i + 1) * P]),
                        rhs=r(wq[k][:, 2 * D + c0 : 2 * D + c0 + cw]),
                        start=(k == 0),
                        stop=(k == DC - 1),
                        skip_group_check=True,
                    )
            vp3 = vpad[i].rearrange("p (h c) -> p h c", c=HD + 1)
            nc.vector.tensor_copy(
                vp3[:, :, 0:HD], ps[:, 0:D].rearrange("p (h c) -> p h c", c=HD)
            )
            # ones columns (f32r memset is invalid ISA): x*0+1 over the
            # just-written first column of each head
            nc.vector.tensor_scalar(
                vp3[:, :, HD : HD + 1],
                vp3[:, :, 0:1],
                0.0,
                1.0,
                mybir.AluOpType.mult,
                mybir.AluOpType.add,
            )

    # ---------------- phase 3: attention ----------------
    with tc.tile_pool(name="att", bufs=2) as att:
        wp = [att.tile([P, D], F32R, tag=f"wp{k}", name=f"wp{k}", bufs=1) for k in range(DC)]
        for k in range(DC):
            nc.sync.dma_start(out=wp[k], in_=r(w_proj[k * P : (k + 1) * P, :]))
        brep = att.tile([P, D], F32, tag="brep", name="brep", bufs=1)
        nc.sync.dma_start(out=brep, in_=b_proj.partition_broadcast(P))

        # Head PAIRS (heads 2p, 2p+1 share the qkT/kT pair tile: head a on
        # partitions 0:64, head b on 64:128). The two S matmuls of a pair are
        # row-tiled (tile_position auto-derived from base_partition) so they
        # run concurrently in the PE array. Software-pipelined over the
        # flattened (pair, key-chunk) sequence: PE order is S(t+1) before
        # O(t) so the PE never waits on exp(t).
        chunks = [(p, m) for p in range(H // 2) for m in range(NT)]
        T = len(chunks)
        oaug = {}
        sps = {}
        epool = {}

        def emit_s(t):
            p, m = chunks[t]
            if m == 0:
                oaug[2 * p] = att_psum.tile([HD + 1, N], F32, tag="oaug", name="oauga")
                oaug[2 * p + 1] = att_psum.tile(
                    [HD + 1, N], F32, tag="oaug", name="oaugb"
                )
            spa = psum.tile([P, N], F32, tag="mm", name="mmpsa")
            spb = psum.tile([P, N], F32, tag="mm", name="mmpsb")
            sps[t] = (spa, spb)
            for n2 in range(2):
                for half, sp in ((0, spa), (1, spb)):
                    row = half * HD
                    kT_h = qkT[6 + p][row : row + HD, :]
                    qT_h = qkT[p][row : row + HD, :]
                    nc.tensor.matmul(
                        sp[:, n2 * 512 : (n2 + 1) * 512],
                        lhsT=r(kT_h[:, m * P : (m + 1) * P]),
                        rhs=r(qT_h[:, n2 * 512 : (n2 + 1) * 512]),
                        start=True,
                        stop=True,
                    )

        def emit_exp(t):
            ea = att.tile([P, N], F32R, tag="e", name="etilea")
            eb = att.tile([P, N], F32R, tag="e", name="etileb")
            epool[t] = (ea, eb)
            nc.scalar.activation(ea, sps[t][0], exp, scale=SCALE)
            nc.scalar.activation(eb, sps[t][1], exp, scale=SCALE)

        def emit_o(t):
            p, m = chunks[t]
            es = epool.pop(t)
            for half in range(2):
                h = 2 * p + half
                vl = vpad[m][:, h * (HD + 1) : (h + 1) * (HD + 1)]
                for n2 in range(2):
                    nc.tensor.matmul(
                        oaug[h][:, n2 * 512 : (n2 + 1) * 512],
                        lhsT=r(vl),
                        rhs=r(es[half][:, n2 * 512 : (n2 + 1) * 512]),
                        start=(m == 0),
                        stop=(m == NT - 1),
                        skip_group_check=True,
                    )
            if m == NT - 1:
                emit_norm(2 * p)
                emit_norm(2 * p + 1)

        def emit_norm(h):
            row = (h % 2) * HD
            oa = oaug.pop(h)
            # Free the PSUM slot fast: copy O to SBUF and spill Z to DRAM.
            osb = att.tile([HD, N], F32, tag="osb", name="osb")
            nc.vector.tensor_copy(osb, oa[0:HD, :])
            zs = att.tile([1, N], F32, tag="zs", name="zs")
            nc.vector.tensor_copy(zs, oa[HD : HD + 1, :])  # DMA can't read PSUM
            zd = zspill.tile([1, N], F32, tag="zd", name="zd", bufs=2)
            nc.sync.dma_start(out=zd, in_=zs)
            # reciprocal is ~6 cyc/element serial per partition: reshape the
            # 1024-long Z row to [128, 8] via DRAM so it runs 128-wide.
            z8 = att.tile([P, N // P], F32, tag="z8", name="z8")
            nc.sync.dma_start(out=z8, in_=zd.rearrange("o (p f) -> (o p) f", p=P))
            r8 = att.tile([P, N // P], F32, tag="r8", name="r8")
            nc.vector.reciprocal(r8, z8)
            rd = zspill.tile([1, N], F32, tag="rd", name="rd", bufs=2)
            nc.sync.dma_start(out=rd.rearrange("o (p f) -> (o p) f", p=P), in_=r8)
            zrep = att.tile([HD, N], F32, tag="zrep", name="zrep")
            nc.sync.dma_start(out=zrep, in_=rd[0, :].partition_broadcast(HD))
            nc.vector.tensor_mul(oT[h // 2][row : row + HD, :], osb, zrep)

        att_psum = ctx.enter_context(tc.tile_pool(name="attps", bufs=2, space="PSUM"))
        zspill = ctx.enter_context(tc.tile_pool(name="zspill", bufs=2, space="DRAM"))
        emit_s(0)
        for t in range(T):
            emit_exp(t)
            if t + 1 < T:
                emit_s(t + 1)
            emit_o(t)

        # ---------------- phase 4: proj ----------------
        for i in range(NT):
            ps = psum.tile([P, N], F32, tag="mm", name="mmps")
            for c0, cw in ((0, 512), (512, 256)):
                for k in range(DC):
                    nc.tensor.matmul(
                        ps[:, c0 : c0 + cw],
                        lhsT=r(oT[k][:, i * P : (i + 1) * P]),
                        rhs=r(wp[k][:, c0 : c0 + cw]),
                        start=(k == 0),
                        stop=(k == DC - 1),
                    )
            yt = att.tile([P, D], F32, tag="y", name="ytile", bufs=3)
            nc.vector.tensor_add(yt, ps[:, 0:D], brep)
            nc.sync.dma_start(out=y[i * P : (i + 1) * P, :], in_=yt)


def build_nc(debug: bool = False):
    nc = bacc.Bacc("TRN2", target_bir_lowering=False, debug=debug, enable_asserts=False)
    x = nc.dram_tensor("x", [N, D], F32, kind="ExternalInput").ap()
    w_qkv = nc.dram_tensor("w_qkv", [D, 3 * D], F32, kind="ExternalInput").ap()
    w_proj = nc.dram_tensor("w_proj", [D, D], F32, kind="ExternalInput").ap()
    b_proj = nc.dram_tensor("b_proj", [D], F32, kind="ExternalInput").ap()
    y = nc.dram_tensor("y", [N, D], F32, kind="ExternalOutput").ap()
    with tile.TileContext(nc) as tc:
        with ExitStack() as ctx:
            build_attention(ctx, tc, x, w_qkv, w_proj, b_proj, y)
    nc.compile()
    return nc


_NC = None


def _get_nc():
    global _NC
    if _NC is None:
        _NC = build_nc()
    return _NC


def kernel(inputs, w_qkv, w_proj, b_proj, _trace=False, **run_kwargs):
    from concourse.bass_utils import run_bass_kernel_spmd

    nc = _get_nc()
    inputs = np.asarray(inputs, dtype=np.float32)
    w_qkv = np.ascontiguousarray(np.asarray(w_qkv, dtype=np.float32))
    w_proj = np.ascontiguousarray(np.asarray(w_proj, dtype=np.float32))
    b_proj = np.ascontiguousarray(np.asarray(b_proj, dtype=np.float32))
    in_maps = [
        {
            "x": np.ascontiguousarray(inputs[i]),
            "w_qkv": w_qkv,
            "w_proj": w_proj,
            "b_proj": b_proj,
        }
        for i in range(NCORES)
    ]
    res = run_bass_kernel_spmd(nc, in_maps, list(range(NCORES)), trace=_trace, **run_kwargs)
    out = np.stack([res.results[i]["y"] for i in range(NCORES)], axis=0)
    if _trace:
        return out, res
    return out


# revision 13
# speedup vs baseline: 1.0745x; 1.0745x over previous
"""Multi-head attention forward on 8 Trainium2 NeuronCores.

Problem: nn_Attention_89060441850459
  inputs [8, 1024, 768] f32, w_qkv [768, 2304], w_proj [768, 768], b_proj [768]
  out = proj(softmax(q k^T / sqrt(64)) v) + b_proj,  H=12 heads, hd=64

Sharding: data parallel over batch — each of the 8 cores computes one batch
element end-to-end; weights replicated. No collectives.

Per-core dataflow (matmul operands in fp16: the fp32 weight path has no
fast/background weight load — ~390-630ns per 512-col matmul; fp16 gets FWL
at 1 cycle/row with fp32 PSUM accumulation; measured end-to-end rel err ~1e-3):

  1. xT[d, n]   = PE-transpose of x[n, d]                       (d-major x)
  2. qkT[m, n]  = w_qkv[:, :1536].T @ xT      (q/k head-dim-major: [1536, 1024])
  3. v[n, c]    = x @ w_qkv[:, 1536:]          (s-major, heads padded with a
                  ones-column per head -> [1024, 12*65] so the PV matmul also
                  produces the softmax denominator for free)
  4. per head h, per key-chunk m (128 rows of kv):
       S^T_m [128k, 1024q] = kT_h[:, m].T-matmul  (lhsT=kT chunk, rhs=qT_h)
       E_m = exp(S^T_m / 8)                        (ACT, PSUM -> SBUF)
       O_aug[65, 1024q] += v_pad_m[:, h].T @ E_m   (accumulated over m in PSUM;
                                                    row 64 = sum_k E = Z)
     then O^T_h = O_aug[0:64] * broadcast(1/Z)     (DVE + DMA partition-bcast)
  5. y = O^T-stacked.T @ w_proj + b_proj           (lhsT = O^T d-major tiles)
"""

import sys

if "/opt/trn_rl_repo" not in sys.path:
    sys.path.insert(0, "/opt/trn_rl_repo")

from contextlib import ExitStack

import numpy as np

import concourse.bass as bass
import concourse.mybir as mybir
import concourse.tile as tile
from concourse import bacc
from concourse.masks import make_identity

B, N, D = 8, 1024, 768
H = 12
HD = D // H  # 64
NCORES = 8
P = 128
NT = N // P  # 8 seq chunks
DC = D // P  # 6 d chunks
F32 = mybir.dt.float32
F32R = mybir.dt.float32r
F16 = mybir.dt.float16
SCALE = HD**-0.5


def r(ap):
    """fp32 -> fp32r view for full-rate PE matmul."""
    return ap.bitcast(F32R)


def build_attention(ctx: ExitStack, tc: "tile.TileContext", x, w_qkv, w_proj, b_proj, y):
    nc = tc.nc
    exp = mybir.ActivationFunctionType.Exp

    perm = ctx.enter_context(tc.tile_pool(name="perm", bufs=1))
    psum = ctx.enter_context(tc.tile_pool(name="psum", bufs=2, space="PSUM"))

    identity = perm.tile([P, P], F32, tag="identity", name="identity")
    make_identity(nc, identity)

    # persistent SBUF arrays
    qkT = [perm.tile([P, N], F16, tag=f"qkT{m}", name=f"qkT{m}") for m in range(12)]  # [0:6]=q, [6:12]=k
    vpad = [perm.tile([P, H * (HD + 1)], F16, tag=f"vpad{i}", name=f"vpad{i}") for i in range(NT)]
    oT = [perm.tile([P, N], F16, tag=f"oT{j}", name=f"oT{j}") for j in range(DC)]

    # ---------------- phase 1+2: xT, v, qkT ----------------
    with tc.tile_pool(name="tmp", bufs=1) as tmp, tc.tile_pool(name="xin", bufs=3) as xin:
        # weights arrive f32 and DMA cannot cast: stage through f32 tiles and
        # cast on the (otherwise idle) scalar engine
        wq = [tmp.tile([P, 3 * D], F16, tag=f"wq{k}", name=f"wq{k}") for k in range(DC)]
        for k in range(DC):
            w32 = xin.tile([P, 3 * D], F32, tag="w32", name="w32")
            nc.sync.dma_start(out=w32, in_=w_qkv[k * P : (k + 1) * P, :])
            nc.scalar.copy(wq[k], w32)
        xT = [tmp.tile([P, N], F16, tag=f"xT{j}", name=f"xT{j}") for j in range(DC)]

        for i in range(NT):
            xt = xin.tile([P, D], F32, tag="x", name="xt")
            nc.sync.dma_start(out=xt, in_=x[i * P : (i + 1) * P, :])
            for j in range(DC):
                pt = psum.tile([P, N], F32, tag="mm", name="mmps")
                nc.tensor.transpose(pt[:, 0:P], xt[:, j * P : (j + 1) * P], identity)
                nc.vector.tensor_copy(xT[j][:, i * P : (i + 1) * P], pt[:, 0:P])

        # v[i][n, c] = sum_k x[n, k] w_qkv[k, 1536+c], written head-padded.
        # v is emitted before qkT so attention (which needs all of v but only
        # one qkT pair per head-pair) can start as early as possible.
        for i in range(NT):
            ps = psum.tile([P, N], F32, tag="mm", name="mmps")
            for k in range(DC):
                for c0, cw in ((0, 512), (512, 256)):
                    nc.tensor.matmul(
                        ps[:, c0 : c0 + cw],
                        lhsT=xT[k][:, i * P : (i + 1) * P],
                        rhs=wq[k][:, 2 * D + c0 : 2 * D + c0 + cw],
                        start=(k == 0),
                        stop=(k == DC - 1),
                        skip_group_check=True,
                    )
            vp3 = vpad[i].rearrange("p (h c) -> p h c", c=HD + 1)
            nc.vector.tensor_copy(
                vp3[:, :, 0:HD], ps[:, 0:D].rearrange("p (h c) -> p h c", c=HD)
            )
            # ones columns (f16 strided memset is invalid ISA): x*0+1 over the
            # just-written first column of each head
            nc.vector.tensor_scalar(
                vp3[:, :, HD : HD + 1],
                vp3[:, :, 0:1],
                0.0,
                1.0,
                mybir.AluOpType.mult,
                mybir.AluOpType.add,
            )

        # qkT[m][dm, n] = sum_k w_qkv[k, m*128+dm] * xT[k, n], emitted in
        # (q_p, k_p) pair order so attention pair p unblocks after 2 tiles
        for m in [t for p in range(6) for t in (p, 6 + p)]:
            ps = psum.tile([P, N], F32, tag="mm", name="mmps")
            for k in range(DC):
                for n2 in range(2):
                    nc.tensor.matmul(
                        ps[:, n2 * 512 : (n2 + 1) * 512],
                        lhsT=wq[k][:, m * P : (m + 1) * P],
                        rhs=xT[k][:, n2 * 512 : (n2 + 1) * 512],
                        start=(k == 0),
                        stop=(k == DC - 1),
                        skip_group_check=True,
                    )
            nc.vector.tensor_copy(qkT[m], ps)

    # ---------------- phase 3: attention ----------------
    with tc.tile_pool(name="att", bufs=2) as att:
        wp = [att.tile([P, D], F16, tag=f"wp{k}", name=f"wp{k}", bufs=1) for k in range(DC)]
        for k in range(DC):
            wp32 = att.tile([P, D], F32, tag="wp32", name="wp32")
            nc.sync.dma_start(out=wp32, in_=w_proj[k * P : (k + 1) * P, :])
            nc.scalar.copy(wp[k], wp32)
        brep = att.tile([P, D], F32, tag="brep", name="brep", bufs=1)
        nc.sync.dma_start(out=brep, in_=b_proj.partition_broadcast(P))

        # Head PAIRS (heads 2p, 2p+1 share the qkT/kT pair tile: head a on
        # partitions 0:64, head b on 64:128). The two S matmuls of a pair are
        # row-tiled (tile_position auto-derived from base_partition) so they
        # run concurrently in the PE array. Software-pipelined over the
        # flattened (pair, key-chunk) sequence: PE order is S(t+1) before
        # O(t) so the PE never waits on exp(t).
        chunks = [(p, m) for p in range(H // 2) for m in range(NT)]
        T = len(chunks)
        oaug = {}
        sps = {}
        epool = {}

        def emit_s(t):
            p, m = chunks[t]
            if m == 0:
                oaug[2 * p] = att_psum.tile([HD + 1, N], F32, tag="oaug", name="oauga")
                oaug[2 * p + 1] = att_psum.tile(
                    [HD + 1, N], F32, tag="oaug", name="oaugb"
                )
            spa = psum.tile([P, N], F32, tag="mm", name="mmpsa")
            spb = psum.tile([P, N], F32, tag="mm", name="mmpsb")
            sps[t] = (spa, spb)
            for n2 in range(2):
                for half, sp in ((0, spa), (1, spb)):
                    row = half * HD
                    kT_h = qkT[6 + p][row : row + HD, :]
                    qT_h = qkT[p][row : row + HD, :]
                    nc.tensor.matmul(
                        sp[:, n2 * 512 : (n2 + 1) * 512],
                        lhsT=kT_h[:, m * P : (m + 1) * P],
                        rhs=qT_h[:, n2 * 512 : (n2 + 1) * 512],
                        start=True,
                        stop=True,
                    )

        def emit_exp(t):
            ea = att.tile([P, N], F16, tag="e", name="etilea")
            eb = att.tile([P, N], F16, tag="e", name="etileb")
            epool[t] = (ea, eb)
            nc.scalar.activation(ea, sps[t][0], exp, scale=SCALE)
            nc.scalar.activation(eb, sps[t][1], exp, scale=SCALE)

        def emit_o(t):
            p, m = chunks[t]
            es = epool.pop(t)
            for half in range(2):
                h = 2 * p + half
                vl = vpad[m][:, h * (HD + 1) : (h + 1) * (HD + 1)]
                for n2 in range(2):
                    nc.tensor.matmul(
                        oaug[h][:, n2 * 512 : (n2 + 1) * 512],
                        lhsT=vl,
                        rhs=es[half][:, n2 * 512 : (n2 + 1) * 512],
                        start=(m == 0),
                        stop=(m == NT - 1),
                        skip_group_check=True,
                    )
            if m == NT - 1:
                emit_norm(2 * p)
                emit_norm(2 * p + 1)

        def emit_norm(h):
            row = (h % 2) * HD
            oa = oaug.pop(h)
            # Free the PSUM slot fast: copy O to SBUF and spill Z to DRAM.
            osb = att.tile([HD, N], F32, tag="osb", name="osb")
            nc.vector.tensor_copy(osb, oa[0:HD, :])
            zs = att.tile([1, N], F32, tag="zs", name="zs")
            nc.vector.tensor_copy(zs, oa[HD : HD + 1, :])  # DMA can't read PSUM
            zd = zspill.tile([1, N], F32, tag="zd", name="zd", bufs=2)
            nc.sync.dma_start(out=zd, in_=zs)
            # reciprocal is ~6 cyc/element serial per partition: reshape the
            # 1024-long Z row to [128, 8] via DRAM so it runs 128-wide.
            z8 = att.tile([P, N // P], F32, tag="z8", name="z8")
            nc.sync.dma_start(out=z8, in_=zd.rearrange("o (p f) -> (o p) f", p=P))
            r8 = att.tile([P, N // P], F32, tag="r8", name="r8")
            nc.vector.reciprocal(r8, z8)
            rd = zspill.tile([1, N], F32, tag="rd", name="rd", bufs=2)
            nc.sync.dma_start(out=rd.rearrange("o (p f) -> (o p) f", p=P), in_=r8)
            zrep = att.tile([HD, N], F32, tag="zrep", name="zrep")
            nc.sync.dma_start(out=zrep, in_=rd[0, :].partition_broadcast(HD))
            nc.vector.tensor_mul(oT[h // 2][row : row + HD, :], osb, zrep)

        att_psum = ctx.enter_context(tc.tile_pool(name="attps", bufs=2, space="PSUM"))
        zspill = ctx.enter_context(tc.tile_pool(name="zspill", bufs=2, space="DRAM"))
        emit_s(0)
        for t in range(T):
            emit_exp(t)
            if t + 1 < T:
                emit_s(t + 1)
            emit_o(t)

        # ---------------- phase 4: proj ----------------
        for i in range(NT):
            ps = psum.tile([P, N], F32, tag="mm", name="mmps")
            for c0, cw in ((0, 512), (512, 256)):
                for k in range(DC):
                    nc.tensor.matmul(
                        ps[:, c0 : c0 + cw],
                        lhsT=oT[k][:, i * P : (i + 1) * P],
                        rhs=wp[k][:, c0 : c0 + cw],
                        start=(k == 0),
                        stop=(k == DC - 1),
                    )
            yt = att.tile([P, D], F32, tag="y", name="ytile", bufs=3)
            nc.vector.tensor_add(yt, ps[:, 0:D], brep)
            nc.sync.dma_start(out=y[i * P : (i + 1) * P, :], in_=yt)


def build_nc(debug: bool = False):
    nc = bacc.Bacc("TRN2", target_bir_lowering=False, debug=debug, enable_asserts=False)
    x = nc.dram_tensor("x", [N, D], F32, kind="ExternalInput").ap()
    w_qkv = nc.dram_tensor("w_qkv", [D, 3 * D], F32, kind="ExternalInput").ap()
    w_proj = nc.dram_tensor("w_proj", [D, D], F32, kind="ExternalInput").ap()
    b_proj = nc.dram_tensor("b_proj", [D], F32, kind="ExternalInput").ap()
    y = nc.dram_tensor("y", [N, D], F32, kind="ExternalOutput").ap()
    with tile.TileContext(nc) as tc:
        with ExitStack() as ctx:
            build_attention(ctx, tc, x, w_qkv, w_proj, b_proj, y)
    nc.compile()
    return nc


_NC = None


def _get_nc():
    global _NC
    if _NC is None:
        _NC = build_nc()
    return _NC


def kernel(inputs, w_qkv, w_proj, b_proj, _trace=False, **run_kwargs):
    from concourse.bass_utils import run_bass_kernel_spmd

    nc = _get_nc()
    inputs = np.asarray(inputs, dtype=np.float32)
    w_qkv = np.ascontiguousarray(np.asarray(w_qkv, dtype=np.float32))
    w_proj = np.ascontiguousarray(np.asarray(w_proj, dtype=np.float32))
    b_proj = np.ascontiguousarray(np.asarray(b_proj, dtype=np.float32))
    in_maps = [
        {
            "x": np.ascontiguousarray(inputs[i]),
            "w_qkv": w_qkv,
            "w_proj": w_proj,
            "b_proj": b_proj,
        }
        for i in range(NCORES)
    ]
    res = run_bass_kernel_spmd(nc, in_maps, list(range(NCORES)), trace=_trace, **run_kwargs)
    out = np.stack([res.results[i]["y"] for i in range(NCORES)], axis=0)
    if _trace:
        return out, res
    return out
